# revision 7
# baseline (speedup 1.0000x reference)
"""Cost-volume kernel for Trainium2 (8 NeuronCores, batch-parallel).

out[b, k, h, w] = (1/(C*81)) * sum_c x[b,c,h,w] * warped[b,c,h+di,w+dj]
for the 81 offsets (di,dj) in [-4,4]^2 (zero-padded), B=8 -> one batch
element per core.

Device-side algorithm (per core):
  - the image is tiled into 8x16 x-tiles (16x16 = 256 tiles). For each
    tile one TensorE matmul computes ALL pairwise channel-dot-products
    between the 128 x-positions (lhsT [C=128, M=128]) and the 16x24
    zero-padded warped window (rhs [C=128, N=384]) -> PSUM [128, 384].
  - warped is staged per tile-COLUMN as SBUF strips [C, 136, 24] (halo
    columns duplicated) so every tile's rhs window is one contiguous
    384-element slice (matmul operands require single-free-dim APs).
  - PSUM blocks are scaled by 1/(C*81) and copied to SBUF (ACT/DVE
    alternating), then DMA'd to a DRAM scratch output [128, 256*384].
  - The 81 shifted dot products per position sit on constant-stride
    "diagonals" of these blocks; on-chip engines cannot express
    partition-correlated free offsets, so the final relayout to
    [81, H, W] is a pure constant-stride (as_strided) view applied while
    unsharding on the host.
"""

import numpy as np

B = 8
C, H, W = 128, 128, 256
R = 4
K = 2 * R + 1  # 9
NOFF = K * K  # 81
TH, TW = 8, 16  # x-tile shape (M = TH*TW = 128)
NH, NW = TH + 2 * R, TW + 2 * R  # warped window 16 x 24
N = NH * NW  # 384
SCALE = 1.0 / (C * NOFF)

_CACHE = {}


def _build_module(c, h, w, th, tw, r, n_cores):
    import concourse.bacc as bacc
    import concourse.mybir as mybir
    import concourse.tile as tile

    k = 2 * r + 1
    nh, nw = th + 2 * r, tw + 2 * r
    n = nh * nw
    nt_h, nt_w = h // th, w // tw
    ntiles = nt_h * nt_w
    ph = h + 2 * r  # padded strip rows
    scale = 1.0 / (c * k * k)
    f32 = mybir.dt.float32

    nc = bacc.Bacc(
        "TRN2", target_bir_lowering=False, debug=False, num_devices=n_cores
    )
    x_d = nc.dram_tensor("x", [c, h, w], f32, kind="ExternalInput").ap()
    w_d = nc.dram_tensor("warped", [c, h, w], f32, kind="ExternalInput").ap()
    out_d = nc.dram_tensor("dump", [128, ntiles * n], f32, kind="ExternalOutput").ap()

    with tile.TileContext(nc) as tc:
        with (
            tc.tile_pool(name="wstrip", bufs=3) as ws_pool,
            tc.tile_pool(name="xstrip", bufs=2) as x_pool,
            tc.tile_pool(name="dump", bufs=2) as dump_pool,
            tc.tile_pool(name="psum", bufs=8, space="PSUM") as psum_pool,
        ):
            t = 0
            for itw in range(nt_w):
                # warped strip for this tile column: padded rows x nw cols
                ws = ws_pool.tile([128, ph * nw], f32)
                ws3 = ws[:].rearrange("p (a b) -> p a b", a=ph)
                nc.vector.memset(ws3[:, 0:r, :], 0.0)
                nc.vector.memset(ws3[:, ph - r : ph, :], 0.0)
                c0 = itw * tw - r  # global col of strip col 0
                cl = max(0, -c0)
                cr = min(nw, w - c0)
                if cl > 0:
                    nc.vector.memset(ws3[:, r : ph - r, 0:cl], 0.0)
                if cr < nw:
                    nc.vector.memset(ws3[:, r : ph - r, cr:nw], 0.0)
                nc.sync.dma_start(
                    out=ws3[:, r : ph - r, cl:cr],
                    in_=w_d[:, :, c0 + cl : c0 + cr],
                )
                # x strip: [c, h rows, tw cols] flat, h-major
                xs = x_pool.tile([128, h * tw], f32)
                nc.sync.dma_start(
                    out=xs, in_=x_d[:, :, itw * tw : (itw + 1) * tw]
                )
                db = dump_pool.tile([128, nt_h * n], f32)
                for ith in range(nt_h):
                    lhsT = xs[:, ith * th * tw : (ith + 1) * th * tw]
                    rhs = ws[:, ith * th * nw : ith * th * nw + n]
                    ps = psum_pool.tile([128, n], f32)
                    nc.tensor.matmul(ps, lhsT, rhs, start=True, stop=True)
                    dst = db[:, ith * n : (ith + 1) * n]
                    # alternate engines so PSUM drain isn't single-engine bound
                    if t % 2 == 0:
                        nc.scalar.mul(dst, ps, scale)
                    else:
                        nc.vector.tensor_scalar_mul(dst, ps, scale)
                    t += 1
                nc.sync.dma_start(
                    out=out_d[:, itw * nt_h * n : (itw + 1) * nt_h * n], in_=db
                )
            assert t == ntiles

    nc.compile()
    return nc


def _extract(dump, h, w, th, tw, r):
    """[128, ntiles*n] f32 scratch -> [81, h, w] via constant-stride view.

    Tile order is tw-major: t = itw*nt_h + ith.
    """
    k = 2 * r + 1
    nh, nw = th + 2 * r, tw + 2 * r
    n = nh * nw
    nt_h, nt_w = h // th, w // tw
    ntiles = nt_h * nt_w
    dmp = np.ascontiguousarray(dump).reshape(128, ntiles, n)
    sm, st, sn = dmp.strides
    # element [m=(hx*tw+wx), t=(itw*nt_h+ith), n=((hx+di)*nw + wx+dj)]
    view = np.lib.stride_tricks.as_strided(
        dmp,
        shape=(k, k, nt_h, th, nt_w, tw),
        strides=(
            nw * sn,            # di
            sn,                 # dj
            st,                 # ith
            tw * sm + nw * sn,  # hx
            nt_h * st,          # itw
            sm + sn,            # wx
        ),
    )
    return np.ascontiguousarray(view).reshape(k * k, h, w)


def kernel(x, warped):
    from concourse import bass_utils

    x = np.asarray(x, dtype=np.float32)
    warped = np.asarray(warped, dtype=np.float32)
    assert x.shape == (B, C, H, W) and warped.shape == (B, C, H, W)

    key = "full"
    if key not in _CACHE:
        _CACHE[key] = _build_module(C, H, W, TH, TW, R, B)
    nc = _CACHE[key]

    in_maps = [
        {
            "x": np.ascontiguousarray(x[b]),
            "warped": np.ascontiguousarray(warped[b]),
        }
        for b in range(B)
    ]
    res = bass_utils.run_bass_kernel_spmd(nc, in_maps, core_ids=list(range(B)))
    global LAST_RESULTS
    LAST_RESULTS = res
    out = np.empty((B, NOFF, H, W), dtype=np.float32)
    for b in range(B):
        out[b] = _extract(res.results[b]["dump"], H, W, TH, TW, R)
    return out


# revision 11
# speedup vs baseline: 1.4066x; 1.4066x over previous
"""Cost-volume kernel for Trainium2 (8 NeuronCores, batch-parallel).

out[b, k, h, w] = (1/(C*81)) * sum_c x[b,c,h,w] * warped[b,c,h+di,w+dj]
for the 81 offsets (di,dj) in [-4,4]^2 (zero-padded), B=8 -> one batch
element per core.

Device-side algorithm (per core):
  - the image is tiled into 8x16 x-tiles (16x16 = 256 tiles). For each
    tile one TensorE matmul computes ALL pairwise channel-dot-products
    between the 128 x-positions (lhsT [C=128, M=128]) and the 16x24
    zero-padded warped window (rhs [C=128, N=384]) -> PSUM [128, 384].
  - warped is staged per tile-COLUMN as SBUF strips [C, 136, 24] (halo
    columns duplicated) so every tile's rhs window is one contiguous
    384-element slice (matmul operands require single-free-dim APs).
  - PSUM blocks are scaled by 1/(C*81) and copied to SBUF (ACT/DVE
    alternating), then DMA'd to a DRAM scratch output [128, 256*384].
  - The 81 shifted dot products per position sit on constant-stride
    "diagonals" of these blocks; on-chip engines cannot express
    partition-correlated free offsets, so the final relayout to
    [81, H, W] is a pure constant-stride (as_strided) view applied while
    unsharding on the host.
"""

import numpy as np

B = 8
C, H, W = 128, 128, 256
R = 4
K = 2 * R + 1  # 9
NOFF = K * K  # 81
TH, TW = 8, 16  # x-tile shape (M = TH*TW = 128)
NH, NW = TH + 2 * R, TW + 2 * R  # warped window 16 x 24
N = NH * NW  # 384
SCALE = 1.0 / (C * NOFF)

# "bf16": bf16 matmul operands + bf16 dump (fp32 PSUM accumulation);
#         halves HBM traffic and avoids the fp32 hi/lo double-pass on PE.
# "f32": exact fp32 end-to-end.
PRECISION = "bf16"

_CACHE = {}


def _build_module(c, h, w, th, tw, r, n_cores, precision):
    import concourse.bacc as bacc
    import concourse.mybir as mybir
    import concourse.tile as tile

    k = 2 * r + 1
    nh, nw = th + 2 * r, tw + 2 * r
    n = nh * nw
    nt_h, nt_w = h // th, w // tw
    ntiles = nt_h * nt_w
    ph = h + 2 * r  # padded strip rows
    scale = 1.0 / (c * k * k)
    dt = mybir.dt.float32 if precision == "f32" else mybir.dt.bfloat16
    f32 = mybir.dt.float32

    nc = bacc.Bacc(
        "TRN2", target_bir_lowering=False, debug=False, num_devices=n_cores
    )
    x_d = nc.dram_tensor("x", [c, h, w], dt, kind="ExternalInput").ap()
    w_d = nc.dram_tensor("warped", [c, h, w], dt, kind="ExternalInput").ap()
    out_d = nc.dram_tensor("dump", [128, ntiles * n], dt, kind="ExternalOutput").ap()

    with tile.TileContext(nc) as tc:
        with (
            tc.tile_pool(name="wstrip", bufs=3) as ws_pool,
            tc.tile_pool(name="xstrip", bufs=2) as x_pool,
            tc.tile_pool(name="dump", bufs=3) as dump_pool,
            tc.tile_pool(name="psum", bufs=8, space="PSUM") as psum_pool,
        ):
            t = 0
            for itw in range(nt_w):
                # warped strip for this tile column: padded rows x nw cols.
                # Loads ride the SP HWDGE ring, x loads the ACT ring, and
                # stores the SWDGE ring so transfers overlap across rings.
                ws = ws_pool.tile([128, ph * nw], dt)
                ws3 = ws[:].rearrange("p (a b) -> p a b", a=ph)
                nc.vector.memset(ws3[:, 0:r, :], 0.0)
                nc.vector.memset(ws3[:, ph - r : ph, :], 0.0)
                c0 = itw * tw - r  # global col of strip col 0
                cl = max(0, -c0)
                cr = min(nw, w - c0)
                if cl > 0:
                    nc.vector.memset(ws3[:, r : ph - r, 0:cl], 0.0)
                if cr < nw:
                    nc.vector.memset(ws3[:, r : ph - r, cr:nw], 0.0)
                nc.sync.dma_start(
                    out=ws3[:, r : ph - r, cl:cr],
                    in_=w_d[:, :, c0 + cl : c0 + cr],
                )
                # x strip: [c, h rows, tw cols] flat, h-major
                xs = x_pool.tile([128, h * tw], dt)
                nc.scalar.dma_start(
                    out=xs, in_=x_d[:, :, itw * tw : (itw + 1) * tw]
                )
                db = dump_pool.tile([128, nt_h * n], dt)
                for ith in range(nt_h):
                    lhsT = xs[:, ith * th * tw : (ith + 1) * th * tw]
                    rhs = ws[:, ith * th * nw : ith * th * nw + n]
                    ps = psum_pool.tile([128, n], f32)
                    nc.tensor.matmul(ps, lhsT, rhs, start=True, stop=True)
                    dst = db[:, ith * n : (ith + 1) * n]
                    # alternate engines so PSUM drain isn't single-engine bound
                    if t % 2 == 0:
                        nc.scalar.mul(dst, ps, scale)
                    else:
                        nc.vector.tensor_scalar_mul(dst, ps, scale)
                    t += 1
                nc.gpsimd.dma_start(
                    out=out_d[:, itw * nt_h * n : (itw + 1) * nt_h * n], in_=db
                )
            assert t == ntiles

    nc.compile()
    return nc


def _extract(dump, h, w, th, tw, r):
    """[128, ntiles*n] f32 scratch -> [81, h, w] via constant-stride view.

    Tile order is tw-major: t = itw*nt_h + ith.
    """
    k = 2 * r + 1
    nh, nw = th + 2 * r, tw + 2 * r
    n = nh * nw
    nt_h, nt_w = h // th, w // tw
    ntiles = nt_h * nt_w
    dmp = np.ascontiguousarray(dump).reshape(128, ntiles, n)
    sm, st, sn = dmp.strides
    # element [m=(hx*tw+wx), t=(itw*nt_h+ith), n=((hx+di)*nw + wx+dj)]
    view = np.lib.stride_tricks.as_strided(
        dmp,
        shape=(k, k, nt_h, th, nt_w, tw),
        strides=(
            nw * sn,            # di
            sn,                 # dj
            st,                 # ith
            tw * sm + nw * sn,  # hx
            nt_h * st,          # itw
            sm + sn,            # wx
        ),
    )
    return np.ascontiguousarray(view).reshape(k * k, h, w).astype(np.float32)


def kernel(x, warped):
    from concourse import bass_utils

    x = np.asarray(x, dtype=np.float32)
    warped = np.asarray(warped, dtype=np.float32)
    assert x.shape == (B, C, H, W) and warped.shape == (B, C, H, W)

    if PRECISION == "bf16":
        import ml_dtypes

        x = x.astype(ml_dtypes.bfloat16)
        warped = warped.astype(ml_dtypes.bfloat16)

    key = PRECISION
    if key not in _CACHE:
        _CACHE[key] = _build_module(C, H, W, TH, TW, R, B, PRECISION)
    nc = _CACHE[key]

    in_maps = [
        {
            "x": np.ascontiguousarray(x[b]),
            "warped": np.ascontiguousarray(warped[b]),
        }
        for b in range(B)
    ]
    res = bass_utils.run_bass_kernel_spmd(nc, in_maps, core_ids=list(range(B)))
    global LAST_RESULTS
    LAST_RESULTS = res
    out = np.empty((B, NOFF, H, W), dtype=np.float32)
    for b in range(B):
        out[b] = _extract(res.results[b]["dump"], H, W, TH, TW, R)
    return out


# revision 14
# speedup vs baseline: 2.2583x; 1.6055x over previous
"""Cost-volume kernel for Trainium2 (8 NeuronCores, batch-parallel).

out[b, k, h, w] = (1/(C*81)) * sum_c x[b,c,h,w] * warped[b,c,h+di,w+dj]
for the 81 offsets (di,dj) in [-4,4]^2 (zero-padded), B=8 -> one batch
element per core.

Device-side algorithm (per core):
  - the image is tiled into 8x16 x-tiles (16x16 = 256 tiles). For each
    tile one TensorE matmul computes ALL pairwise channel-dot-products
    between the 128 x-positions (lhsT [C=128, M=128]) and the 16x24
    zero-padded warped window (rhs [C=128, N=384]) -> PSUM [128, 384].
  - warped is staged per tile-COLUMN as SBUF strips [C, 136, 24] (halo
    columns duplicated) so every tile's rhs window is one contiguous
    384-element slice (matmul operands require single-free-dim APs).
  - PSUM blocks are scaled by 1/(C*81) and copied to SBUF (ACT/DVE
    alternating), then DMA'd to a DRAM scratch output [128, 256*384].
  - The 81 shifted dot products per position sit on constant-stride
    "diagonals" of these blocks; on-chip engines cannot express
    partition-correlated free offsets, so the final relayout to
    [81, H, W] is a pure constant-stride (as_strided) view applied while
    unsharding on the host.
"""

import numpy as np

B = 8
C, H, W = 128, 128, 256
R = 4
K = 2 * R + 1  # 9
NOFF = K * K  # 81
TH, TW = 4, 32  # x-tile shape (M = TH*TW = 128)
NH, NW = TH + 2 * R, TW + 2 * R  # warped window 16 x 24
N = NH * NW  # 384
SCALE = 1.0 / (C * NOFF)

# "bf16": bf16 matmul operands + bf16 dump (fp32 PSUM accumulation);
#         halves HBM traffic and avoids the fp32 hi/lo double-pass on PE.
# "f32": exact fp32 end-to-end.
PRECISION = "bf16"

_CACHE = {}


def _build_module(c, h, w, th, tw, r, n_cores, precision):
    import concourse.bacc as bacc
    import concourse.mybir as mybir
    import concourse.tile as tile

    k = 2 * r + 1
    nh, nw = th + 2 * r, tw + 2 * r
    n = nh * nw
    nt_h, nt_w = h // th, w // tw
    ntiles = nt_h * nt_w
    ph = h + 2 * r  # padded strip rows
    scale = 1.0 / (c * k * k)
    dt = mybir.dt.float32 if precision == "f32" else mybir.dt.bfloat16
    f32 = mybir.dt.float32

    nc = bacc.Bacc(
        "TRN2", target_bir_lowering=False, debug=False, num_devices=n_cores
    )
    x_d = nc.dram_tensor("x", [c, h, w], dt, kind="ExternalInput").ap()
    w_d = nc.dram_tensor("warped", [c, h, w], dt, kind="ExternalInput").ap()
    out_d = nc.dram_tensor("dump", [128, ntiles * n], dt, kind="ExternalOutput").ap()

    with tile.TileContext(nc) as tc:
        with (
            tc.tile_pool(name="wstrip", bufs=3) as ws_pool,
            tc.tile_pool(name="xstrip", bufs=2) as x_pool,
            tc.tile_pool(name="dump", bufs=3) as dump_pool,
            tc.tile_pool(name="psum", bufs=8, space="PSUM") as psum_pool,
        ):
            t = 0
            for itw in range(nt_w):
                # warped strip for this tile column: padded rows x nw cols.
                # Loads ride the SP HWDGE ring, x loads the ACT ring, and
                # stores the SWDGE ring so transfers overlap across rings.
                ws = ws_pool.tile([128, ph * nw], dt)
                ws3 = ws[:].rearrange("p (a b) -> p a b", a=ph)
                nc.vector.memset(ws3[:, 0:r, :], 0.0)
                nc.vector.memset(ws3[:, ph - r : ph, :], 0.0)
                c0 = itw * tw - r  # global col of strip col 0
                cl = max(0, -c0)
                cr = min(nw, w - c0)
                if cl > 0:
                    nc.vector.memset(ws3[:, r : ph - r, 0:cl], 0.0)
                if cr < nw:
                    nc.vector.memset(ws3[:, r : ph - r, cr:nw], 0.0)
                nc.sync.dma_start(
                    out=ws3[:, r : ph - r, cl:cr],
                    in_=w_d[:, :, c0 + cl : c0 + cr],
                )
                # x strip: [c, h rows, tw cols] flat, h-major
                xs = x_pool.tile([128, h * tw], dt)
                nc.scalar.dma_start(
                    out=xs, in_=x_d[:, :, itw * tw : (itw + 1) * tw]
                )
                # store in half-strip groups, round-robin across DMA rings
                assert nt_h % 2 == 0
                half = max(1, nt_h // 2)
                store_engines = [nc.gpsimd, nc.sync, nc.scalar]
                db = None
                for ith in range(nt_h):
                    if ith % half == 0:
                        db = dump_pool.tile([128, half * n], dt)
                    lhsT = xs[:, ith * th * tw : (ith + 1) * th * tw]
                    rhs = ws[:, ith * th * nw : ith * th * nw + n]
                    ps = psum_pool.tile([128, n], f32)
                    nc.tensor.matmul(ps, lhsT, rhs, start=True, stop=True)
                    g = ith % half
                    dst = db[:, g * n : (g + 1) * n]
                    # alternate engines so PSUM drain isn't single-engine bound
                    if t % 2 == 0:
                        nc.scalar.mul(dst, ps, scale)
                    else:
                        nc.vector.tensor_scalar_mul(dst, ps, scale)
                    t += 1
                    if g == half - 1:
                        eng = store_engines[(t // half) % len(store_engines)]
                        base = itw * nt_h * n + (ith - half + 1) * n
                        eng.dma_start(
                            out=out_d[:, base : base + half * n], in_=db
                        )
            assert t == ntiles

    nc.compile()
    return nc


def _extract(dump, h, w, th, tw, r):
    """[128, ntiles*n] f32 scratch -> [81, h, w] via constant-stride view.

    Tile order is tw-major: t = itw*nt_h + ith.
    """
    k = 2 * r + 1
    nh, nw = th + 2 * r, tw + 2 * r
    n = nh * nw
    nt_h, nt_w = h // th, w // tw
    ntiles = nt_h * nt_w
    dmp = np.ascontiguousarray(dump).reshape(128, ntiles, n)
    sm, st, sn = dmp.strides
    # element [m=(hx*tw+wx), t=(itw*nt_h+ith), n=((hx+di)*nw + wx+dj)]
    view = np.lib.stride_tricks.as_strided(
        dmp,
        shape=(k, k, nt_h, th, nt_w, tw),
        strides=(
            nw * sn,            # di
            sn,                 # dj
            st,                 # ith
            tw * sm + nw * sn,  # hx
            nt_h * st,          # itw
            sm + sn,            # wx
        ),
    )
    return np.ascontiguousarray(view).reshape(k * k, h, w).astype(np.float32)


def kernel(x, warped):
    from concourse import bass_utils

    x = np.asarray(x, dtype=np.float32)
    warped = np.asarray(warped, dtype=np.float32)
    assert x.shape == (B, C, H, W) and warped.shape == (B, C, H, W)

    if PRECISION == "bf16":
        import ml_dtypes

        x = x.astype(ml_dtypes.bfloat16)
        warped = warped.astype(ml_dtypes.bfloat16)

    key = PRECISION
    if key not in _CACHE:
        _CACHE[key] = _build_module(C, H, W, TH, TW, R, B, PRECISION)
    nc = _CACHE[key]

    in_maps = [
        {
            "x": np.ascontiguousarray(x[b]),
            "warped": np.ascontiguousarray(warped[b]),
        }
        for b in range(B)
    ]
    res = bass_utils.run_bass_kernel_spmd(nc, in_maps, core_ids=list(range(B)))
    global LAST_RESULTS
    LAST_RESULTS = res
    out = np.empty((B, NOFF, H, W), dtype=np.float32)
    for b in range(B):
        out[b] = _extract(res.results[b]["dump"], H, W, TH, TW, R)
    return out


# revision 17
# speedup vs baseline: 3.7392x; 1.6558x over previous
"""Cost-volume kernel for Trainium2 (8 NeuronCores, batch-parallel).

out[b, k, h, w] = (1/(C*81)) * sum_c x[b,c,h,w] * warped[b,c,h+di,w+dj]
for the 81 offsets (di,dj) in [-4,4]^2 (zero-padded), B=8 -> one batch
element per core.

Device-side algorithm (per core):
  - the image is tiled into 8x16 x-tiles (16x16 = 256 tiles). For each
    tile one TensorE matmul computes ALL pairwise channel-dot-products
    between the 128 x-positions (lhsT [C=128, M=128]) and the 16x24
    zero-padded warped window (rhs [C=128, N=384]) -> PSUM [128, 384].
  - warped is staged per tile-COLUMN as SBUF strips [C, 136, 24] (halo
    columns duplicated) so every tile's rhs window is one contiguous
    384-element slice (matmul operands require single-free-dim APs).
  - PSUM blocks are scaled by 1/(C*81) and copied to SBUF (ACT/DVE
    alternating), then DMA'd to a DRAM scratch output [128, 256*384].
  - The 81 shifted dot products per position sit on constant-stride
    "diagonals" of these blocks; on-chip engines cannot express
    partition-correlated free offsets, so the final relayout to
    [81, H, W] is a pure constant-stride (as_strided) view applied while
    unsharding on the host.
"""

import numpy as np

B = 8
C, H, W = 128, 128, 256
R = 4
K = 2 * R + 1  # 9
NOFF = K * K  # 81
TH, TW = 8, 16  # x-tile shape (M = TH*TW = 128)
NH, NW = TH + 2 * R, TW + 2 * R  # warped window 16 x 24
N = NH * NW  # 384
SCALE = 1.0 / (C * NOFF)

# "bf16": bf16 matmul operands + bf16 dump (fp32 PSUM accumulation);
#         halves HBM traffic and avoids the fp32 hi/lo double-pass on PE.
# "f32": exact fp32 end-to-end.
PRECISION = "bf16"

_CACHE = {}


def _build_module(c, h, w, th, tw, r, n_cores, precision):
    import concourse.bacc as bacc
    import concourse.mybir as mybir
    import concourse.tile as tile

    k = 2 * r + 1
    nh, nw = th + 2 * r, tw + 2 * r
    n = nh * nw
    nt_h, nt_w = h // th, w // tw
    ntiles = nt_h * nt_w
    ph = h + 2 * r  # padded strip rows
    scale = 1.0 / (c * k * k)
    dt = mybir.dt.float32 if precision == "f32" else mybir.dt.bfloat16
    f32 = mybir.dt.float32

    nc = bacc.Bacc(
        "TRN2", target_bir_lowering=False, debug=False, num_devices=n_cores
    )
    # Inputs are pre-stripped host-side (see _host_prep): per tile-column
    # contiguous blocks, warped pre-padded with its 4-wide zero halo and
    # duplicated halo columns. All device DMAs are fully contiguous.
    x_d = nc.dram_tensor("x", [c, nt_w, h * tw], dt, kind="ExternalInput").ap()
    w_d = nc.dram_tensor("warped", [c, nt_w, ph * nw], dt, kind="ExternalInput").ap()
    out_d = nc.dram_tensor("dump", [128, ntiles * n], dt, kind="ExternalOutput").ap()

    with tile.TileContext(nc) as tc:
        with (
            tc.tile_pool(name="wstrip", bufs=3) as ws_pool,
            tc.tile_pool(name="xstrip", bufs=3) as x_pool,
            tc.tile_pool(name="dump", bufs=4) as dump_pool,
            tc.tile_pool(name="psum", bufs=8, space="PSUM") as psum_pool,
        ):
            t = 0
            # loads ride the SP (warped) and ACT (x) HWDGE rings; stores
            # round-robin across SWDGE/SP/ACT so transfers overlap.
            store_engines = [nc.gpsimd, nc.sync, nc.scalar]
            assert nt_h % 2 == 0
            half = max(1, nt_h // 2)
            for itw in range(nt_w):
                ws = ws_pool.tile([128, ph * nw], dt)
                nc.sync.dma_start(out=ws, in_=w_d[:, itw, :])
                xs = x_pool.tile([128, h * tw], dt)
                nc.scalar.dma_start(out=xs, in_=x_d[:, itw, :])
                db = None
                for ith in range(nt_h):
                    if ith % half == 0:
                        db = dump_pool.tile([128, half * n], dt)
                    lhsT = xs[:, ith * th * tw : (ith + 1) * th * tw]
                    rhs = ws[:, ith * th * nw : ith * th * nw + n]
                    ps = psum_pool.tile([128, n], f32)
                    nc.tensor.matmul(ps, lhsT, rhs, start=True, stop=True)
                    g = ith % half
                    dst = db[:, g * n : (g + 1) * n]
                    # alternate engines so PSUM drain isn't single-engine bound
                    if t % 2 == 0:
                        nc.scalar.mul(dst, ps, scale)
                    else:
                        nc.vector.tensor_scalar_mul(dst, ps, scale)
                    t += 1
                    if g == half - 1:
                        eng = store_engines[(t // half) % len(store_engines)]
                        base = itw * nt_h * n + (ith - half + 1) * n
                        eng.dma_start(
                            out=out_d[:, base : base + half * n], in_=db
                        )
            assert t == ntiles

    nc.compile()
    return nc


def _host_prep(x_b, warped_b, h, w, th, tw, r):
    """Re-layout one batch element into per-tile-column strips.

    x: [c, h, w] -> [c, nt_w, h*tw]   (column strips, h-major)
    warped:      -> [c, nt_w, ph*nw]  (pre-padded strips with halo cols)
    """
    c = x_b.shape[0]
    nh, nw = th + 2 * r, tw + 2 * r
    ph = h + 2 * r
    nt_w = w // tw
    xs = np.ascontiguousarray(
        x_b.reshape(c, h, nt_w, tw).transpose(0, 2, 1, 3)
    ).reshape(c, nt_w, h * tw)
    wp = np.zeros((c, ph, w + 2 * r), dtype=x_b.dtype)
    wp[:, r : r + h, r : r + w] = warped_b
    sc, sh, sw = wp.strides
    strips = np.lib.stride_tricks.as_strided(
        wp, shape=(c, nt_w, ph, nw), strides=(sc, tw * sw, sh, sw)
    )
    ws = np.ascontiguousarray(strips).reshape(c, nt_w, ph * nw)
    return xs, ws


def _extract(dump, h, w, th, tw, r):
    """[128, ntiles*n] f32 scratch -> [81, h, w] via constant-stride view.

    Tile order is tw-major: t = itw*nt_h + ith.
    """
    k = 2 * r + 1
    nh, nw = th + 2 * r, tw + 2 * r
    n = nh * nw
    nt_h, nt_w = h // th, w // tw
    ntiles = nt_h * nt_w
    dmp = np.ascontiguousarray(dump).reshape(128, ntiles, n)
    sm, st, sn = dmp.strides
    # element [m=(hx*tw+wx), t=(itw*nt_h+ith), n=((hx+di)*nw + wx+dj)]
    view = np.lib.stride_tricks.as_strided(
        dmp,
        shape=(k, k, nt_h, th, nt_w, tw),
        strides=(
            nw * sn,            # di
            sn,                 # dj
            st,                 # ith
            tw * sm + nw * sn,  # hx
            nt_h * st,          # itw
            sm + sn,            # wx
        ),
    )
    return np.ascontiguousarray(view).reshape(k * k, h, w).astype(np.float32)


def kernel(x, warped):
    from concourse import bass_utils

    x = np.asarray(x, dtype=np.float32)
    warped = np.asarray(warped, dtype=np.float32)
    assert x.shape == (B, C, H, W) and warped.shape == (B, C, H, W)

    if PRECISION == "bf16":
        import ml_dtypes

        x = x.astype(ml_dtypes.bfloat16)
        warped = warped.astype(ml_dtypes.bfloat16)

    key = PRECISION
    if key not in _CACHE:
        _CACHE[key] = _build_module(C, H, W, TH, TW, R, B, PRECISION)
    nc = _CACHE[key]

    in_maps = []
    for b in range(B):
        xs, ws = _host_prep(x[b], warped[b], H, W, TH, TW, R)
        in_maps.append({"x": xs, "warped": ws})
    res = bass_utils.run_bass_kernel_spmd(nc, in_maps, core_ids=list(range(B)))
    global LAST_RESULTS
    LAST_RESULTS = res
    out = np.empty((B, NOFF, H, W), dtype=np.float32)
    for b in range(B):
        out[b] = _extract(res.results[b]["dump"], H, W, TH, TW, R)
    return out


# revision 18
# speedup vs baseline: 4.1249x; 1.1032x over previous
"""Cost-volume kernel for Trainium2 (8 NeuronCores, batch-parallel).

out[b, k, h, w] = (1/(C*81)) * sum_c x[b,c,h,w] * warped[b,c,h+di,w+dj]
for the 81 offsets (di,dj) in [-4,4]^2 (zero-padded), B=8 -> one batch
element per core.

Device-side algorithm (per core):
  - the image is tiled into 8x16 x-tiles (16x16 = 256 tiles). For each
    tile one TensorE matmul computes ALL pairwise channel-dot-products
    between the 128 x-positions (lhsT [C=128, M=128]) and the 16x24
    zero-padded warped window (rhs [C=128, N=384]) -> PSUM [128, 384].
  - warped is staged per tile-COLUMN as SBUF strips [C, 136, 24] (halo
    columns duplicated) so every tile's rhs window is one contiguous
    384-element slice (matmul operands require single-free-dim APs).
  - PSUM blocks are scaled by 1/(C*81) and copied to SBUF (ACT/DVE
    alternating), then DMA'd to a DRAM scratch output [128, 256*384].
  - The 81 shifted dot products per position sit on constant-stride
    "diagonals" of these blocks; on-chip engines cannot express
    partition-correlated free offsets, so the final relayout to
    [81, H, W] is a pure constant-stride (as_strided) view applied while
    unsharding on the host.
"""

import numpy as np

B = 8
C, H, W = 128, 128, 256
R = 4
K = 2 * R + 1  # 9
NOFF = K * K  # 81
TH, TW = 8, 16  # x-tile shape (M = TH*TW = 128)
NH, NW = TH + 2 * R, TW + 2 * R  # warped window 16 x 24
N = NH * NW  # 384
SCALE = 1.0 / (C * NOFF)

# "bf16": bf16 matmul operands + bf16 dump (fp32 PSUM accumulation);
#         halves HBM traffic and avoids the fp32 hi/lo double-pass on PE.
# "f32": exact fp32 end-to-end.
PRECISION = "bf16"

_CACHE = {}


def _build_module(c, h, w, th, tw, r, n_cores, precision):
    import concourse.bacc as bacc
    import concourse.mybir as mybir
    import concourse.tile as tile

    k = 2 * r + 1
    nh, nw = th + 2 * r, tw + 2 * r
    n = nh * nw
    nt_h, nt_w = h // th, w // tw
    ntiles = nt_h * nt_w
    ph = h + 2 * r  # padded strip rows
    scale = 1.0 / (c * k * k)
    dt = mybir.dt.float32 if precision == "f32" else mybir.dt.bfloat16
    f32 = mybir.dt.float32

    nc = bacc.Bacc(
        "TRN2", target_bir_lowering=False, debug=False, num_devices=n_cores
    )
    # Inputs are pre-stripped host-side (see _host_prep): per tile-column
    # contiguous blocks, warped pre-padded with its 4-wide zero halo and
    # duplicated halo columns. All device DMAs are fully contiguous.
    x_d = nc.dram_tensor("x", [c, nt_w, h * tw], dt, kind="ExternalInput").ap()
    w_d = nc.dram_tensor("warped", [c, nt_w, ph * nw], dt, kind="ExternalInput").ap()
    out_d = nc.dram_tensor("dump", [128, ntiles * n], dt, kind="ExternalOutput").ap()

    with tile.TileContext(nc) as tc:
        # PSUM drain is batched: G matmuls land in one bank-padded PSUM
        # group (each MM within its own 2KB bank), drained by a single
        # strided ACT/DVE copy to amortize per-op overhead.
        G = 4 if nt_h % 4 == 0 else (2 if nt_h % 2 == 0 else 1)
        BANK = 512  # fp32 elements per PSUM bank
        assert n <= BANK
        with (
            tc.tile_pool(name="wstrip", bufs=3) as ws_pool,
            tc.tile_pool(name="xstrip", bufs=3) as x_pool,
            tc.tile_pool(name="dump", bufs=4) as dump_pool,
            tc.tile_pool(name="psum", bufs=2, space="PSUM") as psum_pool,
        ):
            t = 0
            gidx = 0
            # loads ride the SP (warped) and ACT (x) HWDGE rings; stores
            # round-robin across SWDGE/SP/ACT so transfers overlap.
            store_engines = [nc.gpsimd, nc.sync, nc.scalar]
            assert nt_h % 2 == 0
            half = max(G, nt_h // 2)
            for itw in range(nt_w):
                ws = ws_pool.tile([128, ph * nw], dt)
                nc.sync.dma_start(out=ws, in_=w_d[:, itw, :])
                xs = x_pool.tile([128, h * tw], dt)
                nc.scalar.dma_start(out=xs, in_=x_d[:, itw, :])
                db = None
                ps = None
                for ith in range(nt_h):
                    if ith % half == 0:
                        db = dump_pool.tile([128, half * n], dt)
                    if ith % G == 0:
                        ps = psum_pool.tile([128, G * BANK], f32)
                    lhsT = xs[:, ith * th * tw : (ith + 1) * th * tw]
                    rhs = ws[:, ith * th * nw : ith * th * nw + n]
                    j = ith % G
                    nc.tensor.matmul(
                        ps[:, j * BANK : j * BANK + n], lhsT, rhs,
                        start=True, stop=True,
                    )
                    t += 1
                    if j == G - 1:
                        # one strided drain for the G tiles
                        src = ps[:].rearrange("p (g x) -> p g x", g=G)[:, :, 0:n]
                        g0 = (ith - G + 1) % half
                        dst = db[:, g0 * n : (g0 + G) * n].rearrange(
                            "p (g x) -> p g x", g=G
                        )
                        if gidx % 2 == 0:
                            nc.scalar.mul(dst, src, scale)
                        else:
                            nc.vector.tensor_scalar_mul(dst, src, scale)
                        gidx += 1
                    if ith % half == half - 1:
                        eng = store_engines[(t // half) % len(store_engines)]
                        base = itw * nt_h * n + (ith - half + 1) * n
                        eng.dma_start(
                            out=out_d[:, base : base + half * n], in_=db
                        )
            assert t == ntiles

    nc.compile()
    return nc


def _host_prep(x_b, warped_b, h, w, th, tw, r):
    """Re-layout one batch element into per-tile-column strips.

    x: [c, h, w] -> [c, nt_w, h*tw]   (column strips, h-major)
    warped:      -> [c, nt_w, ph*nw]  (pre-padded strips with halo cols)
    """
    c = x_b.shape[0]
    nh, nw = th + 2 * r, tw + 2 * r
    ph = h + 2 * r
    nt_w = w // tw
    xs = np.ascontiguousarray(
        x_b.reshape(c, h, nt_w, tw).transpose(0, 2, 1, 3)
    ).reshape(c, nt_w, h * tw)
    wp = np.zeros((c, ph, w + 2 * r), dtype=x_b.dtype)
    wp[:, r : r + h, r : r + w] = warped_b
    sc, sh, sw = wp.strides
    strips = np.lib.stride_tricks.as_strided(
        wp, shape=(c, nt_w, ph, nw), strides=(sc, tw * sw, sh, sw)
    )
    ws = np.ascontiguousarray(strips).reshape(c, nt_w, ph * nw)
    return xs, ws


def _extract(dump, h, w, th, tw, r):
    """[128, ntiles*n] f32 scratch -> [81, h, w] via constant-stride view.

    Tile order is tw-major: t = itw*nt_h + ith.
    """
    k = 2 * r + 1
    nh, nw = th + 2 * r, tw + 2 * r
    n = nh * nw
    nt_h, nt_w = h // th, w // tw
    ntiles = nt_h * nt_w
    dmp = np.ascontiguousarray(dump).reshape(128, ntiles, n)
    sm, st, sn = dmp.strides
    # element [m=(hx*tw+wx), t=(itw*nt_h+ith), n=((hx+di)*nw + wx+dj)]
    view = np.lib.stride_tricks.as_strided(
        dmp,
        shape=(k, k, nt_h, th, nt_w, tw),
        strides=(
            nw * sn,            # di
            sn,                 # dj
            st,                 # ith
            tw * sm + nw * sn,  # hx
            nt_h * st,          # itw
            sm + sn,            # wx
        ),
    )
    return np.ascontiguousarray(view).reshape(k * k, h, w).astype(np.float32)


def kernel(x, warped):
    from concourse import bass_utils

    x = np.asarray(x, dtype=np.float32)
    warped = np.asarray(warped, dtype=np.float32)
    assert x.shape == (B, C, H, W) and warped.shape == (B, C, H, W)

    if PRECISION == "bf16":
        import ml_dtypes

        x = x.astype(ml_dtypes.bfloat16)
        warped = warped.astype(ml_dtypes.bfloat16)

    key = PRECISION
    if key not in _CACHE:
        _CACHE[key] = _build_module(C, H, W, TH, TW, R, B, PRECISION)
    nc = _CACHE[key]

    in_maps = []
    for b in range(B):
        xs, ws = _host_prep(x[b], warped[b], H, W, TH, TW, R)
        in_maps.append({"x": xs, "warped": ws})
    res = bass_utils.run_bass_kernel_spmd(nc, in_maps, core_ids=list(range(B)))
    global LAST_RESULTS
    LAST_RESULTS = res
    out = np.empty((B, NOFF, H, W), dtype=np.float32)
    for b in range(B):
        out[b] = _extract(res.results[b]["dump"], H, W, TH, TW, R)
    return out
